# revision 3
# baseline (speedup 1.0000x reference)
"""RWKV v4 block on 8 TRN2 NeuronCores — fp8 DoubleRow edition.

- Data-parallel over B (core b <- batch b). No collectives.
- Host prep: LN1 + time-shift lerps for k/v/r inputs (fp8 hi+lo split),
  channel-major [C, T] layout, fp8 weight quantization (scale WS).
- Device, two passes over 8 time-chunks of 256:
  Pass 1 (time-mix): k/v/r GEMMs (fp8 DR, input hi/lo), WKV scan (DVE),
    sigmoid-as-tanh gate, Wo GEMM (fp8 DR, weight hi/lo), x2 -> DRAM.
  Pass 2 (FFN): LN2 (f32r stats + Sqrt), lerps, fWk (input+weight split),
    relu^2, fWr (fp8), fWv (weight split), residual -> out.
"""

import numpy as np
import ml_dtypes

B, T, C = 8, 2048, 1024
TC = 256
NCH = T // TC
CB = C // 128
FB = 4 * C // 128
NP = CB // 2          # 4 channel pair-groups
FP = FB // 2          # 16 ffn pair-groups
EPS = 1e-5
WS = 128.0

E4 = ml_dtypes.float8_e4m3
_CACHE = {}

SPLIT_KVR_IN = True
SPLIT_FWK_IN = True
SPLIT_FWK_W = True
SPLIT_WO_W = True
SPLIT_FWV_W = True


def _build():
    import concourse.bass as bass
    import concourse.bacc as bacc
    import concourse.tile as tile
    from concourse import mybir
    import contextlib

    f32 = mybir.dt.float32
    f32r = mybir.dt.float32r
    bf16 = mybir.dt.bfloat16
    f8 = mybir.dt.float8e4
    AF = mybir.ActivationFunctionType
    OP = mybir.AluOpType
    DR = mybir.MatmulPerfMode.DoubleRow

    nc = bacc.Bacc(None, target_bir_lowering=False, debug=False)

    xT = nc.dram_tensor("xT", [C, T], f32, kind="ExternalInput")
    ikh = nc.dram_tensor("ikh", [C, T], f8, kind="ExternalInput")
    ikl = nc.dram_tensor("ikl", [C, T], f8, kind="ExternalInput")
    ivh = nc.dram_tensor("ivh", [C, T], f8, kind="ExternalInput")
    ivl = nc.dram_tensor("ivl", [C, T], f8, kind="ExternalInput")
    irh = nc.dram_tensor("irh", [C, T], f8, kind="ExternalInput")
    Wk8 = nc.dram_tensor("Wk8", [128, CB, C], f8, kind="ExternalInput")
    Wv8 = nc.dram_tensor("Wv8", [128, CB, C], f8, kind="ExternalInput")
    Wr8 = nc.dram_tensor("Wr8", [128, CB, C], f8, kind="ExternalInput")
    Woh = nc.dram_tensor("Woh", [128, CB, C], f8, kind="ExternalInput")
    Wol = nc.dram_tensor("Wol", [128, CB, C], f8, kind="ExternalInput")
    fWkh = nc.dram_tensor("fWkh", [128, CB, 4 * C], f8, kind="ExternalInput")
    fWkl = nc.dram_tensor("fWkl", [128, CB, 4 * C], f8, kind="ExternalInput")
    fWr8 = nc.dram_tensor("fWr8", [128, CB, C], f8, kind="ExternalInput")
    fWvh = nc.dram_tensor("fWvh", [128, FB, C], f8, kind="ExternalInput")
    fWvl = nc.dram_tensor("fWvl", [128, FB, C], f8, kind="ExternalInput")
    euT = nc.dram_tensor("euT", [128, CB], f32, kind="ExternalInput")
    ewT = nc.dram_tensor("ewT", [128, CB], f32, kind="ExternalInput")
    ftkT = nc.dram_tensor("ftkT", [128, CB], f32, kind="ExternalInput")
    ftrT = nc.dram_tensor("ftrT", [128, CB], f32, kind="ExternalInput")
    bfrT = nc.dram_tensor("bfrT", [128, CB], f32, kind="ExternalInput")
    bfkT = nc.dram_tensor("bfkT", [128, FB], f32, kind="ExternalInput")
    ones_in = nc.dram_tensor("ones128", [128], f32r, kind="ExternalInput")
    outT = nc.dram_tensor("outT", [C, T], f32, kind="ExternalOutput")

    IWS = 1.0 / WS
    IWSL = 1.0 / (WS * 16.0)

    def dmaN(out_t, in_ap, parts=1, eng=None):
        e = eng or nc.sync
        M = out_t.shape[1]
        step = max(1, M // parts)
        for i in range(0, M, step):
            j = min(i + step, M)
            e.dma_start(out=out_t[:, i:j, :], in_=in_ap[:, i:j, :])

    xre = xT.rearrange("(a p) t -> p a t", p=128)
    oure = outT.rearrange("(a p) t -> p a t", p=128)

    with tile.TileContext(nc) as tc:
        with contextlib.ExitStack() as ctx:
            cp = ctx.enter_context(tc.tile_pool(name="cp", bufs=1))
            dramp = ctx.enter_context(tc.tile_pool(name="dram", bufs=1, space="DRAM"))

            eu_t = cp.tile([128, CB], f32)
            nc.sync.dma_start(out=eu_t, in_=euT[:, :])
            ew_t = cp.tile([128, CB], f32)
            nc.sync.dma_start(out=ew_t, in_=ewT[:, :])
            ftk_t = cp.tile([128, CB], f32)
            nc.sync.dma_start(out=ftk_t, in_=ftkT[:, :])
            ftr_t = cp.tile([128, CB], f32)
            nc.sync.dma_start(out=ftr_t, in_=ftrT[:, :])
            bfr_t = cp.tile([128, CB], f32)
            nc.sync.dma_start(out=bfr_t, in_=bfrT[:, :])
            bfk_t = cp.tile([128, FB], f32)
            nc.sync.dma_start(out=bfk_t, in_=bfkT[:, :])
            ones_k = cp.tile([128, 1], f32r)
            nc.sync.dma_start(out=ones_k, in_=ones_in.rearrange("(p o) -> p o", o=1))
            ones_b = cp.tile([1, 128], f32r)
            nc.sync.dma_start(out=ones_b, in_=ones_in.rearrange("(o p) -> o p", o=1))

            carryA = cp.tile([128, CB, 1], f32)
            carryB = cp.tile([128, CB, 1], f32)
            carryG = cp.tile([128, CB, 1], bf16)
            nc.vector.memset(carryA, 0.0)
            nc.vector.memset(carryB, 0.0)
            nc.vector.memset(carryG, 0.0)

            x2d = dramp.tile([NCH, 128, CB, TC], f32)

            def gemm(ps, w_t, co, rhs_hi, rhs_lo=None, w_lo=None, npair=NP):
                csl = slice(co * 128, (co + 1) * 128)
                total = npair * (1 + (rhs_lo is not None) + (w_lo is not None))
                n = 0
                for j in range(npair):
                    wsl = w_t[:, 2 * j:2 * j + 2, csl]
                    rsl = rhs_hi[:, 2 * j:2 * j + 2, :]
                    n += 1
                    nc.tensor.matmul(ps, wsl, rsl, start=(n == 1),
                                     stop=(n == total), perf_mode=DR)
                    if rhs_lo is not None:
                        n += 1
                        nc.tensor.matmul(ps, wsl, rhs_lo[:, 2 * j:2 * j + 2, :],
                                         start=False, stop=(n == total),
                                         perf_mode=DR)
                    if w_lo is not None:
                        n += 1
                        nc.tensor.matmul(ps, w_lo[:, 2 * j:2 * j + 2, csl], rsl,
                                         start=False, stop=(n == total),
                                         perf_mode=DR)

            # ======================= Pass 1: time-mix =======================
            with contextlib.ExitStack() as p1:
                wp = p1.enter_context(tc.tile_pool(name="wp1", bufs=1))
                act = p1.enter_context(tc.tile_pool(name="act1", bufs=2))
                wkv = p1.enter_context(tc.tile_pool(name="wkv", bufs=1))
                ekp = p1.enter_context(tc.tile_pool(name="ekp", bufs=2))
                ps_mm = p1.enter_context(tc.tile_pool(name="ps1", bufs=6, space="PSUM"))

                wk_t = wp.tile([128, CB, C], f8, tag="wk")
                dmaN(wk_t, Wk8[:, :, :])
                wv_t = wp.tile([128, CB, C], f8, tag="wv")
                dmaN(wv_t, Wv8[:, :, :])
                wr_t = wp.tile([128, CB, C], f8, tag="wr")
                dmaN(wr_t, Wr8[:, :, :])
                woh_t = wp.tile([128, CB, C], f8, tag="woh")
                dmaN(woh_t, Woh[:, :, :])
                wol_t = None
                if SPLIT_WO_W:
                    wol_t = wp.tile([128, CB, C], f8, tag="wol")
                    dmaN(wol_t, Wol[:, :, :])

                for ic in range(NCH):
                    tsl = slice(ic * TC, ic * TC + TC)
                    x_t = act.tile([128, CB, TC], f32, tag="x")
                    dmaN(x_t, xre[:, :, tsl], eng=nc.gpsimd)
                    kh_t = act.tile([128, CB, TC], f8, tag="kh")
                    dmaN(kh_t, ikh.rearrange("(a p) t -> p a t", p=128)[:, :, tsl], eng=nc.gpsimd)
                    vh_t = act.tile([128, CB, TC], f8, tag="vh")
                    dmaN(vh_t, ivh.rearrange("(a p) t -> p a t", p=128)[:, :, tsl], eng=nc.gpsimd)
                    rh_t = act.tile([128, CB, TC], f8, tag="rh")
                    dmaN(rh_t, irh.rearrange("(a p) t -> p a t", p=128)[:, :, tsl], eng=nc.gpsimd)
                    kl_t = vl_t = None
                    if SPLIT_KVR_IN:
                        kl_t = act.tile([128, CB, TC], f8, tag="kl")
                        dmaN(kl_t, ikl.rearrange("(a p) t -> p a t", p=128)[:, :, tsl], eng=nc.gpsimd)
                        vl_t = act.tile([128, CB, TC], f8, tag="vl")
                        dmaN(vl_t, ivl.rearrange("(a p) t -> p a t", p=128)[:, :, tsl], eng=nc.gpsimd)

                    ek = ekp.tile([128, CB, TC], f32, tag="ek")
                    ekv = ekp.tile([128, CB, TC], f32, tag="ekv")
                    th = ekp.tile([128, CB, TC], bf16, tag="th")
                    for co in range(CB):
                        ps_k = ps_mm.tile([128, TC], f32, tag="mm")
                        gemm(ps_k, wk_t, co, kh_t, rhs_lo=kl_t)
                        nc.scalar.activation(out=ek[:, co, :], in_=ps_k,
                                             func=AF.Exp, scale=IWS)
                    for co in range(CB):
                        ps_v = ps_mm.tile([128, TC], f32, tag="mm")
                        gemm(ps_v, wv_t, co, vh_t, rhs_lo=vl_t)
                        nc.vector.scalar_tensor_tensor(
                            out=ekv[:, co, :], in0=ps_v, scalar=IWS,
                            in1=ek[:, co, :], op0=OP.mult, op1=OP.mult)
                    for co in range(CB):
                        ps_r = ps_mm.tile([128, TC], f32, tag="mm")
                        gemm(ps_r, wr_t, co, rh_t)
                        nc.scalar.activation(out=th[:, co, :], in_=ps_r,
                                             func=AF.Tanh, scale=0.5 * IWS)

                    A_t = wkv.tile([128, CB, TC + 1], f32, tag="A")
                    B_t = wkv.tile([128, CB, TC + 1], f32, tag="B")
                    nc.vector.tensor_copy(out=A_t[:, :, 0:1], in_=carryA)
                    nc.vector.tensor_copy(out=B_t[:, :, 0:1], in_=carryB)
                    for cb in range(CB):
                        ewj = ew_t[:, cb:cb + 1]
                        ew_b = bass.AP(tensor=ewj.tensor, offset=ewj.offset,
                                       ap=[ewj.ap[0], [0, TC]])
                        nc.vector.tensor_tensor_scan(
                            out=A_t[:, cb, 1:TC + 1], data0=ew_b,
                            data1=ekv[:, cb, :], initial=A_t[:, cb, 0:1],
                            op0=OP.mult, op1=OP.add)
                        nc.vector.tensor_tensor_scan(
                            out=B_t[:, cb, 1:TC + 1], data0=ew_b,
                            data1=ek[:, cb, :], initial=B_t[:, cb, 0:1],
                            op0=OP.mult, op1=OP.add)
                    nc.vector.tensor_copy(out=carryA, in_=A_t[:, :, TC:TC + 1])
                    nc.vector.tensor_copy(out=carryB, in_=B_t[:, :, TC:TC + 1])

                    num = wkv.tile([128, CB, TC], f32, tag="num")
                    den = wkv.tile([128, CB, TC], f32, tag="den")
                    for cb in range(CB):
                        nc.vector.scalar_tensor_tensor(
                            out=num[:, cb, :], in0=ekv[:, cb, :],
                            scalar=eu_t[:, cb:cb + 1], in1=A_t[:, cb, 0:TC],
                            op0=OP.mult, op1=OP.add)
                        nc.vector.scalar_tensor_tensor(
                            out=den[:, cb, :], in0=ek[:, cb, :],
                            scalar=eu_t[:, cb:cb + 1], in1=B_t[:, cb, 0:TC],
                            op0=OP.mult, op1=OP.add)
                    rden = wkv.tile([128, CB, TC], f32, tag="rden")
                    nc.vector.reciprocal_approx_fast(out=rden, in_=den)
                    t1 = wkv.tile([128, CB, TC], f32, tag="t1")
                    nc.vector.scalar_tensor_tensor(
                        out=t1, in0=th, scalar=1.0, in1=num,
                        op0=OP.add, op1=OP.mult)
                    y8 = wkv.tile([128, CB, TC], f8, tag="y8")
                    nc.vector.tensor_mul(y8, t1, rden)

                    x2_t = act.tile([128, CB, TC], f32, tag="x2")
                    for co in range(CB):
                        ps_o = ps_mm.tile([128, TC], f32, tag="mm")
                        gemm(ps_o, woh_t, co, y8, w_lo=wol_t)
                        nc.vector.scalar_tensor_tensor(
                            out=x2_t[:, co, :], in0=ps_o, scalar=IWS,
                            in1=x_t[:, co, :], op0=OP.mult, op1=OP.add)
                    dmaN(x2d[ic], x2_t, eng=nc.gpsimd)

            # ========================= Pass 2: FFN ==========================
            with contextlib.ExitStack() as p2:
                wp = p2.enter_context(tc.tile_pool(name="wp2", bufs=1))
                act = p2.enter_context(tc.tile_pool(name="act2", bufs=1))
                op2 = p2.enter_context(tc.tile_pool(name="op2", bufs=1))
                ffn = p2.enter_context(tc.tile_pool(name="ffn", bufs=1))
                rows = p2.enter_context(tc.tile_pool(name="rows", bufs=1))
                ps_f = p2.enter_context(tc.tile_pool(name="ps2", bufs=5, space="PSUM"))
                ps_st = p2.enter_context(tc.tile_pool(name="ps_st", bufs=1, space="PSUM"))
                ps_bc = p2.enter_context(tc.tile_pool(name="ps_bc", bufs=1, space="PSUM"))

                fwkh_t = wp.tile([128, CB, 4 * C], f8, tag="fwkh")
                dmaN(fwkh_t, fWkh[:, :, :], parts=8)
                fwkl_t = None
                if SPLIT_FWK_W:
                    fwkl_t = wp.tile([128, CB, 4 * C], f8, tag="fwkl")
                    dmaN(fwkl_t, fWkl[:, :, :], parts=8)
                fwr_t = wp.tile([128, CB, C], f8, tag="fwr")
                dmaN(fwr_t, fWr8[:, :, :])
                fwvh_t = wp.tile([128, FB, C], f8, tag="fwvh")
                dmaN(fwvh_t, fWvh[:, :, :], parts=8)
                fwvl_t = None
                if SPLIT_FWV_W:
                    fwvl_t = wp.tile([128, FB, C], f8, tag="fwvl")
                    dmaN(fwvl_t, fWvl[:, :, :], parts=8)

                for ic in range(NCH):
                    tsl = slice(ic * TC, ic * TC + TC)
                    x2_t = act.tile([128, CB, TC], f32, tag="x2")
                    dmaN(x2_t, x2d[ic], eng=nc.gpsimd)

                    sq = ffn.tile([128, CB, TC], f32r, tag="sq")
                    nc.scalar.activation(out=sq, in_=x2_t, func=AF.Square)
                    st = ps_st.tile([1, 2 * TC], f32, tag="st")
                    x2r = x2_t.bitcast(f32r)
                    for cb in range(CB):
                        nc.tensor.matmul(st[:, 0:TC], ones_k, x2r[:, cb, :],
                                         start=(cb == 0), stop=(cb == CB - 1))
                    for cb in range(CB):
                        nc.tensor.matmul(st[:, TC:2 * TC], ones_k, sq[:, cb, :],
                                         start=(cb == 0), stop=(cb == CB - 1))
                    rw = rows.tile([1, 2 * TC], f32r, tag="rw")
                    rwf = rw.bitcast(f32)
                    tmp = rows.tile([1, 2 * TC], f32, tag="tmp")
                    nc.vector.tensor_scalar_mul(rw[:, 0:TC], st[:, 0:TC], -1.0 / C)
                    nc.vector.tensor_mul(tmp[:, 0:TC], rwf[:, 0:TC], rwf[:, 0:TC])
                    nc.vector.scalar_tensor_tensor(
                        out=tmp[:, TC:2 * TC], in0=st[:, TC:2 * TC],
                        scalar=1.0 / C, in1=tmp[:, 0:TC],
                        op0=OP.mult, op1=OP.subtract)
                    nc.vector.tensor_scalar_add(tmp[:, 0:TC], tmp[:, TC:2 * TC], EPS)
                    nc.vector.reciprocal(out=tmp[:, TC:2 * TC], in_=tmp[:, 0:TC])
                    nc.scalar.activation(out=rw[:, TC:2 * TC],
                                         in_=tmp[:, TC:2 * TC], func=AF.Sqrt)
                    mb = ps_bc.tile([128, TC], f32, tag="mb")
                    nc.tensor.matmul(mb, ones_b, rw[:, 0:TC])
                    rb = ps_bc.tile([128, TC], f32, tag="rb")
                    nc.tensor.matmul(rb, ones_b, rw[:, TC:2 * TC])

                    def bmid(ap):
                        return bass.AP(tensor=ap.tensor, offset=ap.offset,
                                       ap=[ap.ap[0], [0, CB], ap.ap[1]])
                    nc.vector.tensor_add(sq, x2_t, bmid(mb))
                    g_t = ffn.tile([128, CB, TC + 1], bf16, tag="g")
                    nc.vector.tensor_copy(out=g_t[:, :, 0:1], in_=carryG)
                    nc.vector.tensor_mul(g_t[:, :, 1:TC + 1], sq.bitcast(f32),
                                         bmid(rb))
                    nc.vector.tensor_copy(out=carryG, in_=g_t[:, :, TC:TC + 1])

                    d2 = ffn.tile([128, CB, TC], bf16, tag="d2")
                    nc.vector.tensor_sub(d2, g_t[:, :, 1:TC + 1], g_t[:, :, 0:TC])
                    inf_b = ffn.tile([128, CB, TC], bf16, tag="infb")
                    infr8 = ffn.tile([128, CB, TC], f8, tag="infr")
                    for cb in range(CB):
                        nc.vector.scalar_tensor_tensor(
                            out=inf_b[:, cb, :], in0=d2[:, cb, :],
                            scalar=ftk_t[:, cb:cb + 1], in1=g_t[:, cb, 0:TC],
                            op0=OP.mult, op1=OP.add)
                        nc.vector.scalar_tensor_tensor(
                            out=infr8[:, cb, :], in0=d2[:, cb, :],
                            scalar=ftr_t[:, cb:cb + 1], in1=g_t[:, cb, 0:TC],
                            op0=OP.mult, op1=OP.add)
                    infh = ffn.tile([128, CB, TC], f8, tag="infh")
                    nc.scalar.activation(out=infh, in_=inf_b, func=AF.Copy)
                    infl = None
                    if SPLIT_FWK_IN:
                        infl = ffn.tile([128, CB, TC], f8, tag="infl")
                        nc.vector.tensor_sub(infl, inf_b, infh)

                    kk8 = ffn.tile([128, FB, TC], f8, tag="kk8")
                    for cog in range(4):
                        rt = ffn.tile([128, CB, TC], bf16, tag="rt")
                        for ci in range(CB):
                            co = cog * CB + ci
                            ps_z = ps_f.tile([128, TC], f32, tag="fz")
                            gemm(ps_z, fwkh_t, co, infh, rhs_lo=infl,
                                 w_lo=fwkl_t)
                            nc.scalar.activation(out=rt[:, ci, :], in_=ps_z,
                                                 func=AF.Relu, scale=IWS,
                                                 bias=bfk_t[:, co:co + 1])
                        nc.vector.tensor_mul(
                            kk8[:, cog * CB:(cog + 1) * CB, :], rt, rt)

                    th2 = ffn.tile([128, CB, TC], bf16, tag="th2")
                    for co in range(CB):
                        ps_r2 = ps_f.tile([128, TC], f32, tag="fz")
                        gemm(ps_r2, fwr_t, co, infr8)
                        nc.scalar.activation(out=th2[:, co, :], in_=ps_r2,
                                             func=AF.Tanh, scale=0.5 * IWS,
                                             bias=bfr_t[:, co:co + 1])

                    out_t = op2.tile([128, CB, TC], f32, tag="out")
                    for co in range(CB):
                        ps_v2 = ps_f.tile([128, TC], f32, tag="fz")
                        gemm(ps_v2, fwvh_t, co, kk8, w_lo=fwvl_t, npair=FP)
                        t2 = ffn.tile([128, TC], f32, tag="t2")
                        nc.vector.scalar_tensor_tensor(
                            out=t2, in0=th2[:, co, :], scalar=1.0,
                            in1=ps_v2, op0=OP.add, op1=OP.mult)
                        nc.vector.scalar_tensor_tensor(
                            out=out_t[:, co, :], in0=t2, scalar=IWS,
                            in1=x2_t[:, co, :], op0=OP.mult, op1=OP.add)
                    dmaN(oure[:, :, tsl], out_t, eng=nc.gpsimd)

    nc.finalize()
    return nc


def _q8(x, s=1.0):
    return (np.asarray(x, np.float32) * s).astype(E4)


def _pack_w(W):
    """[C, M] -> [128, CB, M] with a = c // 128, p = c % 128."""
    Cin, M = W.shape
    return np.ascontiguousarray(W.reshape(Cin // 128, 128, M).transpose(1, 0, 2))


def _prep_maps(inputs):
    x = np.asarray(inputs["x"], np.float32)
    ln1_g = np.asarray(inputs["ln1_g"], np.float32)
    ln1_b = np.asarray(inputs["ln1_b"], np.float32)
    ln2_g = np.asarray(inputs["ln2_g"], np.float32)
    ln2_b = np.asarray(inputs["ln2_b"], np.float32)
    tmk = np.asarray(inputs["tmk"], np.float32)
    tmv = np.asarray(inputs["tmv"], np.float32)
    tmr = np.asarray(inputs["tmr"], np.float32)
    ftmk = np.asarray(inputs["ftmk"], np.float32)
    ftmr = np.asarray(inputs["ftmr"], np.float32)

    m = x.mean(-1, keepdims=True)
    v = np.square(x - m).mean(-1, keepdims=True)
    h = ((x - m) / np.sqrt(v + EPS)) * ln1_g + ln1_b
    hh = np.concatenate([np.zeros((B, 1, C), np.float32), h[:, :-1]], 1)
    ink = h * tmk + hh * (1 - tmk)
    inv = h * tmv + hh * (1 - tmv)
    inr = h * tmr + hh * (1 - tmr)

    def split(t):
        hi = _q8(t)
        lo = _q8(t - hi.astype(np.float32))
        return hi, lo

    ikh_, ikl_ = split(ink)
    ivh_, ivl_ = split(inv)
    irh_ = _q8(inr)

    Wk = np.asarray(inputs["Wk"], np.float32)
    Wv = np.asarray(inputs["Wv"], np.float32) * 0.5
    Wr = np.asarray(inputs["Wr"], np.float32)
    Wo = np.asarray(inputs["Wo"], np.float32)
    fWk = np.asarray(inputs["fWk"], np.float32) * ln2_g[:, None]
    fWr = np.asarray(inputs["fWr"], np.float32) * ln2_g[:, None]
    fWv = np.asarray(inputs["fWv"], np.float32) * 0.5

    def wsplit(W):
        hi = _q8(W, WS)
        lo = _q8(W - hi.astype(np.float32) / WS, WS)
        return hi, lo

    Woh_, Wol_ = wsplit(Wo)
    fWkh_, fWkl_ = wsplit(fWk)
    fWvh_, fWvl_ = wsplit(fWv)

    ew = np.exp(-np.exp(np.asarray(inputs["time_decay"], np.float32)))
    eu = np.exp(np.asarray(inputs["time_first"], np.float32))
    bias_fk = ln2_b @ np.asarray(inputs["fWk"], np.float32)
    bias_fr = ln2_b @ np.asarray(inputs["fWr"], np.float32)

    def rows128(a):
        return np.ascontiguousarray(a.reshape(-1, 128).T.astype(np.float32))

    common = {
        "Wk8": _pack_w(_q8(Wk, WS)), "Wv8": _pack_w(_q8(Wv, WS)),
        "Wr8": _pack_w(_q8(Wr, WS)),
        "Woh": _pack_w(Woh_), "Wol": _pack_w(Wol_),
        "fWkh": _pack_w(fWkh_), "fWkl": _pack_w(fWkl_),
        "fWr8": _pack_w(_q8(fWr, WS)),
        "fWvh": _pack_w(fWvh_), "fWvl": _pack_w(fWvl_),
        "euT": rows128(eu), "ewT": rows128(ew),
        "ftkT": rows128(ftmk), "ftrT": rows128(ftmr),
        "bfrT": rows128(0.5 * bias_fr), "bfkT": rows128(bias_fk),
        "ones128": np.ones(128, np.float32),
    }
    maps = []
    for b in range(B):
        maps.append({**common,
                     "xT": np.ascontiguousarray(x[b].T),
                     "ikh": np.ascontiguousarray(ikh_[b].T),
                     "ikl": np.ascontiguousarray(ikl_[b].T),
                     "ivh": np.ascontiguousarray(ivh_[b].T),
                     "ivl": np.ascontiguousarray(ivl_[b].T),
                     "irh": np.ascontiguousarray(irh_[b].T)})
    return maps


def get_nc():
    if "nc" not in _CACHE:
        _CACHE["nc"] = _build()
    return _CACHE["nc"]


def kernel(**inputs):
    from concourse.bass_utils import run_bass_kernel_spmd
    nc = get_nc()
    in_maps = _prep_maps(inputs)
    res = run_bass_kernel_spmd(nc, in_maps, core_ids=list(range(B)))
    return np.stack([np.ascontiguousarray(r["outT"].T) for r in res.results])


# revision 4
# speedup vs baseline: 1.2927x; 1.2927x over previous
"""RWKV v4 block on 8 TRN2 NeuronCores — fp8 DoubleRow edition.

- Data-parallel over B (core b <- batch b). No collectives.
- Host prep: LN1 + time-shift lerps for k/v/r inputs (fp8 hi+lo split),
  channel-major [C, T] layout, fp8 weight quantization (scale WS).
- Device, two passes over 8 time-chunks of 256:
  Pass 1 (time-mix): k/v/r GEMMs (fp8 DR, input hi/lo), WKV scan (DVE),
    sigmoid-as-tanh gate, Wo GEMM (fp8 DR, weight hi/lo), x2 -> DRAM.
  Pass 2 (FFN): LN2 (f32r stats + Sqrt), lerps, fWk (input+weight split),
    relu^2, fWr (fp8), fWv (weight split), residual -> out.
"""

import numpy as np
import ml_dtypes

B, T, C = 8, 2048, 1024
TC = 256
NCH = T // TC
CB = C // 128
FB = 4 * C // 128
NP = CB // 2          # 4 channel pair-groups
FP = FB // 2          # 16 ffn pair-groups
EPS = 1e-5
WS = 128.0

E4 = ml_dtypes.float8_e4m3
_CACHE = {}

SPLIT_KVR_IN = True
SPLIT_FWK_IN = True
SPLIT_FWK_W = True
SPLIT_WO_W = True
SPLIT_FWV_W = True


def _build():
    import concourse.bass as bass
    import concourse.bacc as bacc
    import concourse.tile as tile
    from concourse import mybir
    import contextlib

    f32 = mybir.dt.float32
    f32r = mybir.dt.float32r
    bf16 = mybir.dt.bfloat16
    f8 = mybir.dt.float8e4
    AF = mybir.ActivationFunctionType
    OP = mybir.AluOpType
    DR = mybir.MatmulPerfMode.DoubleRow

    nc = bacc.Bacc(None, target_bir_lowering=False, debug=False)

    xT = nc.dram_tensor("xT", [C, T], f32, kind="ExternalInput")
    ikh = nc.dram_tensor("ikh", [C, T], f8, kind="ExternalInput")
    ikl = nc.dram_tensor("ikl", [C, T], f8, kind="ExternalInput")
    ivh = nc.dram_tensor("ivh", [C, T], f8, kind="ExternalInput")
    ivl = nc.dram_tensor("ivl", [C, T], f8, kind="ExternalInput")
    irh = nc.dram_tensor("irh", [C, T], f8, kind="ExternalInput")
    Wk8 = nc.dram_tensor("Wk8", [128, CB, C], f8, kind="ExternalInput")
    Wv8 = nc.dram_tensor("Wv8", [128, CB, C], f8, kind="ExternalInput")
    Wr8 = nc.dram_tensor("Wr8", [128, CB, C], f8, kind="ExternalInput")
    Woh = nc.dram_tensor("Woh", [128, CB, C], f8, kind="ExternalInput")
    Wol = nc.dram_tensor("Wol", [128, CB, C], f8, kind="ExternalInput")
    fWkh = nc.dram_tensor("fWkh", [128, CB, 4 * C], f8, kind="ExternalInput")
    fWkl = nc.dram_tensor("fWkl", [128, CB, 4 * C], f8, kind="ExternalInput")
    fWr8 = nc.dram_tensor("fWr8", [128, CB, C], f8, kind="ExternalInput")
    fWvh = nc.dram_tensor("fWvh", [128, FB, C], f8, kind="ExternalInput")
    fWvl = nc.dram_tensor("fWvl", [128, FB, C], f8, kind="ExternalInput")
    euT = nc.dram_tensor("euT", [128, CB], f32, kind="ExternalInput")
    ewT = nc.dram_tensor("ewT", [128, CB], f32, kind="ExternalInput")
    ftkT = nc.dram_tensor("ftkT", [128, CB], f32, kind="ExternalInput")
    ftrT = nc.dram_tensor("ftrT", [128, CB], f32, kind="ExternalInput")
    bfrT = nc.dram_tensor("bfrT", [128, CB], f32, kind="ExternalInput")
    bfkT = nc.dram_tensor("bfkT", [128, FB], f32, kind="ExternalInput")
    ones_in = nc.dram_tensor("ones128", [128], f32r, kind="ExternalInput")
    outT = nc.dram_tensor("outT", [C, T], f32, kind="ExternalOutput")

    IWS = 1.0 / WS
    IWSL = 1.0 / (WS * 16.0)

    def dmaN(out_t, in_ap, parts=4, eng=None):
        e = eng or nc.sync
        M = out_t.shape[1]
        step = max(1, M // parts)
        for i in range(0, M, step):
            j = min(i + step, M)
            e.dma_start(out=out_t[:, i:j, :], in_=in_ap[:, i:j, :])

    xre = xT.rearrange("(a p) t -> p a t", p=128)
    oure = outT.rearrange("(a p) t -> p a t", p=128)

    with tile.TileContext(nc) as tc:
        with contextlib.ExitStack() as ctx:
            cp = ctx.enter_context(tc.tile_pool(name="cp", bufs=1))
            dramp = ctx.enter_context(tc.tile_pool(name="dram", bufs=1, space="DRAM"))

            eu_t = cp.tile([128, CB], f32)
            nc.sync.dma_start(out=eu_t, in_=euT[:, :])
            ew_t = cp.tile([128, CB], f32)
            nc.sync.dma_start(out=ew_t, in_=ewT[:, :])
            ftk_t = cp.tile([128, CB], f32)
            nc.sync.dma_start(out=ftk_t, in_=ftkT[:, :])
            ftr_t = cp.tile([128, CB], f32)
            nc.sync.dma_start(out=ftr_t, in_=ftrT[:, :])
            bfr_t = cp.tile([128, CB], f32)
            nc.sync.dma_start(out=bfr_t, in_=bfrT[:, :])
            bfk_t = cp.tile([128, FB], f32)
            nc.sync.dma_start(out=bfk_t, in_=bfkT[:, :])
            ones_k = cp.tile([128, 1], f32r)
            nc.sync.dma_start(out=ones_k, in_=ones_in.rearrange("(p o) -> p o", o=1))
            ones_b = cp.tile([1, 128], f32r)
            nc.sync.dma_start(out=ones_b, in_=ones_in.rearrange("(o p) -> o p", o=1))

            carryA = cp.tile([128, CB, 1], f32)
            carryB = cp.tile([128, CB, 1], f32)
            carryG = cp.tile([128, CB, 1], bf16)
            nc.vector.memset(carryA, 0.0)
            nc.vector.memset(carryB, 0.0)
            nc.vector.memset(carryG, 0.0)

            x2d = dramp.tile([NCH, 128, CB, TC], f32)

            def gemm(ps, w_t, co, rhs_hi, rhs_lo=None, w_lo=None, npair=NP):
                csl = slice(co * 128, (co + 1) * 128)
                total = npair * (1 + (rhs_lo is not None) + (w_lo is not None))
                n = 0
                for j in range(npair):
                    wsl = w_t[:, 2 * j:2 * j + 2, csl]
                    rsl = rhs_hi[:, 2 * j:2 * j + 2, :]
                    n += 1
                    nc.tensor.matmul(ps, wsl, rsl, start=(n == 1),
                                     stop=(n == total), perf_mode=DR)
                    if rhs_lo is not None:
                        n += 1
                        nc.tensor.matmul(ps, wsl, rhs_lo[:, 2 * j:2 * j + 2, :],
                                         start=False, stop=(n == total),
                                         perf_mode=DR)
                    if w_lo is not None:
                        n += 1
                        nc.tensor.matmul(ps, w_lo[:, 2 * j:2 * j + 2, csl], rsl,
                                         start=False, stop=(n == total),
                                         perf_mode=DR)

            # ======================= Pass 1: time-mix =======================
            with contextlib.ExitStack() as p1:
                wp = p1.enter_context(tc.tile_pool(name="wp1", bufs=1))
                act = p1.enter_context(tc.tile_pool(name="act1", bufs=2))
                wkv = p1.enter_context(tc.tile_pool(name="wkv", bufs=1))
                ps_mm = p1.enter_context(tc.tile_pool(name="ps1", bufs=6, space="PSUM"))

                wk_t = wp.tile([128, CB, C], f8, tag="wk")
                dmaN(wk_t, Wk8[:, :, :])
                wv_t = wp.tile([128, CB, C], f8, tag="wv")
                dmaN(wv_t, Wv8[:, :, :])
                wr_t = wp.tile([128, CB, C], f8, tag="wr")
                dmaN(wr_t, Wr8[:, :, :])
                woh_t = wp.tile([128, CB, C], f8, tag="woh")
                dmaN(woh_t, Woh[:, :, :])
                wol_t = None
                if SPLIT_WO_W:
                    wol_t = wp.tile([128, CB, C], f8, tag="wol")
                    dmaN(wol_t, Wol[:, :, :])

                for ic in range(NCH):
                    tsl = slice(ic * TC, ic * TC + TC)
                    x_t = act.tile([128, CB, TC], f32, tag="x")
                    dmaN(x_t, xre[:, :, tsl])
                    kh_t = act.tile([128, CB, TC], f8, tag="kh")
                    dmaN(kh_t, ikh.rearrange("(a p) t -> p a t", p=128)[:, :, tsl], 2)
                    vh_t = act.tile([128, CB, TC], f8, tag="vh")
                    dmaN(vh_t, ivh.rearrange("(a p) t -> p a t", p=128)[:, :, tsl], 2)
                    rh_t = act.tile([128, CB, TC], f8, tag="rh")
                    dmaN(rh_t, irh.rearrange("(a p) t -> p a t", p=128)[:, :, tsl], 2)
                    kl_t = vl_t = None
                    if SPLIT_KVR_IN:
                        kl_t = act.tile([128, CB, TC], f8, tag="kl")
                        dmaN(kl_t, ikl.rearrange("(a p) t -> p a t", p=128)[:, :, tsl], 2)
                        vl_t = act.tile([128, CB, TC], f8, tag="vl")
                        dmaN(vl_t, ivl.rearrange("(a p) t -> p a t", p=128)[:, :, tsl], 2)

                    ek = wkv.tile([128, CB, TC], f32, tag="ek")
                    ekv = wkv.tile([128, CB, TC], f32, tag="ekv")
                    th = wkv.tile([128, CB, TC], bf16, tag="th")
                    for co in range(CB):
                        ps_k = ps_mm.tile([128, TC], f32, tag="mm")
                        gemm(ps_k, wk_t, co, kh_t, rhs_lo=kl_t)
                        nc.scalar.activation(out=ek[:, co, :], in_=ps_k,
                                             func=AF.Exp, scale=IWS)
                    for co in range(CB):
                        ps_v = ps_mm.tile([128, TC], f32, tag="mm")
                        gemm(ps_v, wv_t, co, vh_t, rhs_lo=vl_t)
                        nc.vector.scalar_tensor_tensor(
                            out=ekv[:, co, :], in0=ps_v, scalar=IWS,
                            in1=ek[:, co, :], op0=OP.mult, op1=OP.mult)
                    for co in range(CB):
                        ps_r = ps_mm.tile([128, TC], f32, tag="mm")
                        gemm(ps_r, wr_t, co, rh_t)
                        nc.scalar.activation(out=th[:, co, :], in_=ps_r,
                                             func=AF.Tanh, scale=0.5 * IWS)

                    A_t = wkv.tile([128, CB, TC + 1], f32, tag="A")
                    B_t = wkv.tile([128, CB, TC + 1], f32, tag="B")
                    nc.vector.tensor_copy(out=A_t[:, :, 0:1], in_=carryA)
                    nc.vector.tensor_copy(out=B_t[:, :, 0:1], in_=carryB)
                    for cb in range(CB):
                        ewj = ew_t[:, cb:cb + 1]
                        ew_b = bass.AP(tensor=ewj.tensor, offset=ewj.offset,
                                       ap=[ewj.ap[0], [0, TC]])
                        nc.vector.tensor_tensor_scan(
                            out=A_t[:, cb, 1:TC + 1], data0=ew_b,
                            data1=ekv[:, cb, :], initial=A_t[:, cb, 0:1],
                            op0=OP.mult, op1=OP.add)
                        nc.vector.tensor_tensor_scan(
                            out=B_t[:, cb, 1:TC + 1], data0=ew_b,
                            data1=ek[:, cb, :], initial=B_t[:, cb, 0:1],
                            op0=OP.mult, op1=OP.add)
                    nc.vector.tensor_copy(out=carryA, in_=A_t[:, :, TC:TC + 1])
                    nc.vector.tensor_copy(out=carryB, in_=B_t[:, :, TC:TC + 1])

                    num = wkv.tile([128, CB, TC], f32, tag="num")
                    den = wkv.tile([128, CB, TC], f32, tag="den")
                    for cb in range(CB):
                        nc.vector.scalar_tensor_tensor(
                            out=num[:, cb, :], in0=ekv[:, cb, :],
                            scalar=eu_t[:, cb:cb + 1], in1=A_t[:, cb, 0:TC],
                            op0=OP.mult, op1=OP.add)
                        nc.vector.scalar_tensor_tensor(
                            out=den[:, cb, :], in0=ek[:, cb, :],
                            scalar=eu_t[:, cb:cb + 1], in1=B_t[:, cb, 0:TC],
                            op0=OP.mult, op1=OP.add)
                    rden = wkv.tile([128, CB, TC], f32, tag="rden")
                    nc.vector.reciprocal_approx_fast(out=rden, in_=den)
                    t1 = wkv.tile([128, CB, TC], f32, tag="t1")
                    nc.vector.scalar_tensor_tensor(
                        out=t1, in0=th, scalar=1.0, in1=num,
                        op0=OP.add, op1=OP.mult)
                    y8 = wkv.tile([128, CB, TC], f8, tag="y8")
                    nc.vector.tensor_mul(y8, t1, rden)

                    x2_t = act.tile([128, CB, TC], f32, tag="x2")
                    for co in range(CB):
                        ps_o = ps_mm.tile([128, TC], f32, tag="mm")
                        gemm(ps_o, woh_t, co, y8, w_lo=wol_t)
                        nc.vector.scalar_tensor_tensor(
                            out=x2_t[:, co, :], in0=ps_o, scalar=IWS,
                            in1=x_t[:, co, :], op0=OP.mult, op1=OP.add)
                    dmaN(x2d[ic], x2_t)

            # ========================= Pass 2: FFN ==========================
            with contextlib.ExitStack() as p2:
                wp = p2.enter_context(tc.tile_pool(name="wp2", bufs=1))
                act = p2.enter_context(tc.tile_pool(name="act2", bufs=1))
                op2 = p2.enter_context(tc.tile_pool(name="op2", bufs=1))
                ffn = p2.enter_context(tc.tile_pool(name="ffn", bufs=1))
                rows = p2.enter_context(tc.tile_pool(name="rows", bufs=1))
                ps_f = p2.enter_context(tc.tile_pool(name="ps2", bufs=5, space="PSUM"))
                ps_st = p2.enter_context(tc.tile_pool(name="ps_st", bufs=1, space="PSUM"))
                ps_bc = p2.enter_context(tc.tile_pool(name="ps_bc", bufs=1, space="PSUM"))

                fwkh_t = wp.tile([128, CB, 4 * C], f8, tag="fwkh")
                dmaN(fwkh_t, fWkh[:, :, :], parts=8)
                fwkl_t = None
                if SPLIT_FWK_W:
                    fwkl_t = wp.tile([128, CB, 4 * C], f8, tag="fwkl")
                    dmaN(fwkl_t, fWkl[:, :, :], parts=8)
                fwr_t = wp.tile([128, CB, C], f8, tag="fwr")
                dmaN(fwr_t, fWr8[:, :, :])
                fwvh_t = wp.tile([128, FB, C], f8, tag="fwvh")
                dmaN(fwvh_t, fWvh[:, :, :], parts=8)
                fwvl_t = None
                if SPLIT_FWV_W:
                    fwvl_t = wp.tile([128, FB, C], f8, tag="fwvl")
                    dmaN(fwvl_t, fWvl[:, :, :], parts=8)

                for ic in range(NCH):
                    tsl = slice(ic * TC, ic * TC + TC)
                    x2_t = act.tile([128, CB, TC], f32, tag="x2")
                    dmaN(x2_t, x2d[ic])

                    sq = ffn.tile([128, CB, TC], f32r, tag="sq")
                    nc.scalar.activation(out=sq, in_=x2_t, func=AF.Square)
                    st = ps_st.tile([1, 2 * TC], f32, tag="st")
                    x2r = x2_t.bitcast(f32r)
                    for cb in range(CB):
                        nc.tensor.matmul(st[:, 0:TC], ones_k, x2r[:, cb, :],
                                         start=(cb == 0), stop=(cb == CB - 1))
                    for cb in range(CB):
                        nc.tensor.matmul(st[:, TC:2 * TC], ones_k, sq[:, cb, :],
                                         start=(cb == 0), stop=(cb == CB - 1))
                    rw = rows.tile([1, 2 * TC], f32r, tag="rw")
                    rwf = rw.bitcast(f32)
                    tmp = rows.tile([1, 2 * TC], f32, tag="tmp")
                    nc.vector.tensor_scalar_mul(rw[:, 0:TC], st[:, 0:TC], -1.0 / C)
                    nc.vector.tensor_mul(tmp[:, 0:TC], rwf[:, 0:TC], rwf[:, 0:TC])
                    nc.vector.scalar_tensor_tensor(
                        out=tmp[:, TC:2 * TC], in0=st[:, TC:2 * TC],
                        scalar=1.0 / C, in1=tmp[:, 0:TC],
                        op0=OP.mult, op1=OP.subtract)
                    nc.vector.tensor_scalar_add(tmp[:, 0:TC], tmp[:, TC:2 * TC], EPS)
                    nc.vector.reciprocal(out=tmp[:, TC:2 * TC], in_=tmp[:, 0:TC])
                    nc.scalar.activation(out=rw[:, TC:2 * TC],
                                         in_=tmp[:, TC:2 * TC], func=AF.Sqrt)
                    mb = ps_bc.tile([128, TC], f32, tag="mb")
                    nc.tensor.matmul(mb, ones_b, rw[:, 0:TC])
                    rb = ps_bc.tile([128, TC], f32, tag="rb")
                    nc.tensor.matmul(rb, ones_b, rw[:, TC:2 * TC])

                    def bmid(ap):
                        return bass.AP(tensor=ap.tensor, offset=ap.offset,
                                       ap=[ap.ap[0], [0, CB], ap.ap[1]])
                    nc.vector.tensor_add(sq, x2_t, bmid(mb))
                    g_t = ffn.tile([128, CB, TC + 1], bf16, tag="g")
                    nc.vector.tensor_copy(out=g_t[:, :, 0:1], in_=carryG)
                    nc.vector.tensor_mul(g_t[:, :, 1:TC + 1], sq.bitcast(f32),
                                         bmid(rb))
                    nc.vector.tensor_copy(out=carryG, in_=g_t[:, :, TC:TC + 1])

                    d2 = ffn.tile([128, CB, TC], bf16, tag="d2")
                    nc.vector.tensor_sub(d2, g_t[:, :, 1:TC + 1], g_t[:, :, 0:TC])
                    inf_b = ffn.tile([128, CB, TC], bf16, tag="infb")
                    infr8 = ffn.tile([128, CB, TC], f8, tag="infr")
                    for cb in range(CB):
                        nc.vector.scalar_tensor_tensor(
                            out=inf_b[:, cb, :], in0=d2[:, cb, :],
                            scalar=ftk_t[:, cb:cb + 1], in1=g_t[:, cb, 0:TC],
                            op0=OP.mult, op1=OP.add)
                        nc.vector.scalar_tensor_tensor(
                            out=infr8[:, cb, :], in0=d2[:, cb, :],
                            scalar=ftr_t[:, cb:cb + 1], in1=g_t[:, cb, 0:TC],
                            op0=OP.mult, op1=OP.add)
                    infh = ffn.tile([128, CB, TC], f8, tag="infh")
                    nc.scalar.activation(out=infh, in_=inf_b, func=AF.Copy)
                    infl = None
                    if SPLIT_FWK_IN:
                        infl = ffn.tile([128, CB, TC], f8, tag="infl")
                        nc.vector.tensor_sub(infl, inf_b, infh)

                    kk8 = ffn.tile([128, FB, TC], f8, tag="kk8")
                    for cog in range(4):
                        rt = ffn.tile([128, CB, TC], bf16, tag="rt")
                        for ci in range(CB):
                            co = cog * CB + ci
                            ps_z = ps_f.tile([128, TC], f32, tag="fz")
                            gemm(ps_z, fwkh_t, co, infh, rhs_lo=infl,
                                 w_lo=fwkl_t)
                            nc.scalar.activation(out=rt[:, ci, :], in_=ps_z,
                                                 func=AF.Relu, scale=IWS,
                                                 bias=bfk_t[:, co:co + 1])
                        nc.vector.tensor_mul(
                            kk8[:, cog * CB:(cog + 1) * CB, :], rt, rt)

                    th2 = ffn.tile([128, CB, TC], bf16, tag="th2")
                    for co in range(CB):
                        ps_r2 = ps_f.tile([128, TC], f32, tag="fz")
                        gemm(ps_r2, fwr_t, co, infr8)
                        nc.scalar.activation(out=th2[:, co, :], in_=ps_r2,
                                             func=AF.Tanh, scale=0.5 * IWS,
                                             bias=bfr_t[:, co:co + 1])

                    out_t = op2.tile([128, CB, TC], f32, tag="out")
                    for co in range(CB):
                        ps_v2 = ps_f.tile([128, TC], f32, tag="fz")
                        gemm(ps_v2, fwvh_t, co, kk8, w_lo=fwvl_t, npair=FP)
                        t2 = ffn.tile([128, TC], f32, tag="t2")
                        nc.vector.scalar_tensor_tensor(
                            out=t2, in0=th2[:, co, :], scalar=1.0,
                            in1=ps_v2, op0=OP.add, op1=OP.mult)
                        nc.vector.scalar_tensor_tensor(
                            out=out_t[:, co, :], in0=t2, scalar=IWS,
                            in1=x2_t[:, co, :], op0=OP.mult, op1=OP.add)
                    dmaN(oure[:, :, tsl], out_t)

    nc.finalize()
    return nc


def _q8(x, s=1.0):
    return (np.asarray(x, np.float32) * s).astype(E4)


def _pack_w(W):
    """[C, M] -> [128, CB, M] with a = c // 128, p = c % 128."""
    Cin, M = W.shape
    return np.ascontiguousarray(W.reshape(Cin // 128, 128, M).transpose(1, 0, 2))


def _prep_maps(inputs):
    x = np.asarray(inputs["x"], np.float32)
    ln1_g = np.asarray(inputs["ln1_g"], np.float32)
    ln1_b = np.asarray(inputs["ln1_b"], np.float32)
    ln2_g = np.asarray(inputs["ln2_g"], np.float32)
    ln2_b = np.asarray(inputs["ln2_b"], np.float32)
    tmk = np.asarray(inputs["tmk"], np.float32)
    tmv = np.asarray(inputs["tmv"], np.float32)
    tmr = np.asarray(inputs["tmr"], np.float32)
    ftmk = np.asarray(inputs["ftmk"], np.float32)
    ftmr = np.asarray(inputs["ftmr"], np.float32)

    m = x.mean(-1, keepdims=True)
    v = np.square(x - m).mean(-1, keepdims=True)
    h = ((x - m) / np.sqrt(v + EPS)) * ln1_g + ln1_b
    hh = np.concatenate([np.zeros((B, 1, C), np.float32), h[:, :-1]], 1)
    ink = h * tmk + hh * (1 - tmk)
    inv = h * tmv + hh * (1 - tmv)
    inr = h * tmr + hh * (1 - tmr)

    def split(t):
        hi = _q8(t)
        lo = _q8(t - hi.astype(np.float32))
        return hi, lo

    ikh_, ikl_ = split(ink)
    ivh_, ivl_ = split(inv)
    irh_ = _q8(inr)

    Wk = np.asarray(inputs["Wk"], np.float32)
    Wv = np.asarray(inputs["Wv"], np.float32) * 0.5
    Wr = np.asarray(inputs["Wr"], np.float32)
    Wo = np.asarray(inputs["Wo"], np.float32)
    fWk = np.asarray(inputs["fWk"], np.float32) * ln2_g[:, None]
    fWr = np.asarray(inputs["fWr"], np.float32) * ln2_g[:, None]
    fWv = np.asarray(inputs["fWv"], np.float32) * 0.5

    def wsplit(W):
        hi = _q8(W, WS)
        lo = _q8(W - hi.astype(np.float32) / WS, WS)
        return hi, lo

    Woh_, Wol_ = wsplit(Wo)
    fWkh_, fWkl_ = wsplit(fWk)
    fWvh_, fWvl_ = wsplit(fWv)

    ew = np.exp(-np.exp(np.asarray(inputs["time_decay"], np.float32)))
    eu = np.exp(np.asarray(inputs["time_first"], np.float32))
    bias_fk = ln2_b @ np.asarray(inputs["fWk"], np.float32)
    bias_fr = ln2_b @ np.asarray(inputs["fWr"], np.float32)

    def rows128(a):
        return np.ascontiguousarray(a.reshape(-1, 128).T.astype(np.float32))

    common = {
        "Wk8": _pack_w(_q8(Wk, WS)), "Wv8": _pack_w(_q8(Wv, WS)),
        "Wr8": _pack_w(_q8(Wr, WS)),
        "Woh": _pack_w(Woh_), "Wol": _pack_w(Wol_),
        "fWkh": _pack_w(fWkh_), "fWkl": _pack_w(fWkl_),
        "fWr8": _pack_w(_q8(fWr, WS)),
        "fWvh": _pack_w(fWvh_), "fWvl": _pack_w(fWvl_),
        "euT": rows128(eu), "ewT": rows128(ew),
        "ftkT": rows128(ftmk), "ftrT": rows128(ftmr),
        "bfrT": rows128(0.5 * bias_fr), "bfkT": rows128(bias_fk),
        "ones128": np.ones(128, np.float32),
    }
    maps = []
    for b in range(B):
        maps.append({**common,
                     "xT": np.ascontiguousarray(x[b].T),
                     "ikh": np.ascontiguousarray(ikh_[b].T),
                     "ikl": np.ascontiguousarray(ikl_[b].T),
                     "ivh": np.ascontiguousarray(ivh_[b].T),
                     "ivl": np.ascontiguousarray(ivl_[b].T),
                     "irh": np.ascontiguousarray(irh_[b].T)})
    return maps


def get_nc():
    if "nc" not in _CACHE:
        _CACHE["nc"] = _build()
    return _CACHE["nc"]


def kernel(**inputs):
    from concourse.bass_utils import run_bass_kernel_spmd
    nc = get_nc()
    in_maps = _prep_maps(inputs)
    res = run_bass_kernel_spmd(nc, in_maps, core_ids=list(range(B)))
    return np.stack([np.ascontiguousarray(r["outT"].T) for r in res.results])


# revision 5
# speedup vs baseline: 1.2940x; 1.0010x over previous
"""RWKV v4 block on 8 TRN2 NeuronCores — fp8 DoubleRow edition.

- Data-parallel over B (core b <- batch b). No collectives.
- Host prep: LN1 + time-shift lerps for k/v/r inputs (fp8 hi+lo split),
  channel-major [C, T] layout, fp8 weight quantization (scale WS).
- Device, two passes over 8 time-chunks of 256:
  Pass 1 (time-mix): k/v/r GEMMs (fp8 DR, input hi/lo), WKV scan (DVE),
    sigmoid-as-tanh gate, Wo GEMM (fp8 DR, weight hi/lo), x2 -> DRAM.
  Pass 2 (FFN): LN2 (f32r stats + Sqrt), lerps, fWk (input+weight split),
    relu^2, fWr (fp8), fWv (weight split), residual -> out.
"""

import numpy as np
import ml_dtypes

B, T, C = 8, 2048, 1024
TC = 256
NCH = T // TC
CB = C // 128
FB = 4 * C // 128
NP = CB // 2          # 4 channel pair-groups
FP = FB // 2          # 16 ffn pair-groups
EPS = 1e-5
WS = 128.0

E4 = ml_dtypes.float8_e4m3
_CACHE = {}

SPLIT_KVR_IN = True
SPLIT_FWK_IN = True
SPLIT_FWK_W = True
SPLIT_WO_W = True
SPLIT_FWV_W = True


def _build():
    import concourse.bass as bass
    import concourse.bacc as bacc
    import concourse.tile as tile
    from concourse import mybir
    import contextlib

    f32 = mybir.dt.float32
    f32r = mybir.dt.float32r
    bf16 = mybir.dt.bfloat16
    f8 = mybir.dt.float8e4
    AF = mybir.ActivationFunctionType
    OP = mybir.AluOpType
    DR = mybir.MatmulPerfMode.DoubleRow

    nc = bacc.Bacc(None, target_bir_lowering=False, debug=False)

    xT = nc.dram_tensor("xT", [C, T], f32, kind="ExternalInput")
    ikh = nc.dram_tensor("ikh", [C, T], f8, kind="ExternalInput")
    ikl = nc.dram_tensor("ikl", [C, T], f8, kind="ExternalInput")
    ivh = nc.dram_tensor("ivh", [C, T], f8, kind="ExternalInput")
    ivl = nc.dram_tensor("ivl", [C, T], f8, kind="ExternalInput")
    irh = nc.dram_tensor("irh", [C, T], f8, kind="ExternalInput")
    Wk8 = nc.dram_tensor("Wk8", [128, CB, C], f8, kind="ExternalInput")
    Wv8 = nc.dram_tensor("Wv8", [128, CB, C], f8, kind="ExternalInput")
    Wr8 = nc.dram_tensor("Wr8", [128, CB, C], f8, kind="ExternalInput")
    Woh = nc.dram_tensor("Woh", [128, CB, C], f8, kind="ExternalInput")
    Wol = nc.dram_tensor("Wol", [128, CB, C], f8, kind="ExternalInput")
    fWkh = nc.dram_tensor("fWkh", [128, CB, 4 * C], f8, kind="ExternalInput")
    fWkl = nc.dram_tensor("fWkl", [128, CB, 4 * C], f8, kind="ExternalInput")
    fWr8 = nc.dram_tensor("fWr8", [128, CB, C], f8, kind="ExternalInput")
    fWvh = nc.dram_tensor("fWvh", [128, FB, C], f8, kind="ExternalInput")
    fWvl = nc.dram_tensor("fWvl", [128, FB, C], f8, kind="ExternalInput")
    euT = nc.dram_tensor("euT", [128, CB], f32, kind="ExternalInput")
    ewT = nc.dram_tensor("ewT", [128, CB], f32, kind="ExternalInput")
    ftkT = nc.dram_tensor("ftkT", [128, CB], f32, kind="ExternalInput")
    ftrT = nc.dram_tensor("ftrT", [128, CB], f32, kind="ExternalInput")
    bfrT = nc.dram_tensor("bfrT", [128, CB], f32, kind="ExternalInput")
    bfkT = nc.dram_tensor("bfkT", [128, FB], f32, kind="ExternalInput")
    ones_in = nc.dram_tensor("ones128", [128], f32r, kind="ExternalInput")
    outT = nc.dram_tensor("outT", [C, T], f32, kind="ExternalOutput")

    IWS = 1.0 / WS
    IWSL = 1.0 / (WS * 16.0)

    def dmaN(out_t, in_ap, parts=4, eng=None):
        e = eng or nc.sync
        M = out_t.shape[1]
        step = max(1, M // parts)
        for i in range(0, M, step):
            j = min(i + step, M)
            e.dma_start(out=out_t[:, i:j, :], in_=in_ap[:, i:j, :])

    xre = xT.rearrange("(a p) t -> p a t", p=128)
    oure = outT.rearrange("(a p) t -> p a t", p=128)

    with tile.TileContext(nc) as tc:
        with contextlib.ExitStack() as ctx:
            cp = ctx.enter_context(tc.tile_pool(name="cp", bufs=1))
            dramp = ctx.enter_context(tc.tile_pool(name="dram", bufs=1, space="DRAM"))

            eu_t = cp.tile([128, CB], f32)
            nc.sync.dma_start(out=eu_t, in_=euT[:, :])
            ew_t = cp.tile([128, CB], f32)
            nc.sync.dma_start(out=ew_t, in_=ewT[:, :])
            ftk_t = cp.tile([128, CB], f32)
            nc.sync.dma_start(out=ftk_t, in_=ftkT[:, :])
            ftr_t = cp.tile([128, CB], f32)
            nc.sync.dma_start(out=ftr_t, in_=ftrT[:, :])
            bfr_t = cp.tile([128, CB], f32)
            nc.sync.dma_start(out=bfr_t, in_=bfrT[:, :])
            bfk_t = cp.tile([128, FB], f32)
            nc.sync.dma_start(out=bfk_t, in_=bfkT[:, :])
            ones_k = cp.tile([128, 1], f32r)
            nc.sync.dma_start(out=ones_k, in_=ones_in.rearrange("(p o) -> p o", o=1))
            ones_b = cp.tile([1, 128], f32r)
            nc.sync.dma_start(out=ones_b, in_=ones_in.rearrange("(o p) -> o p", o=1))

            carryA = cp.tile([128, CB, 1], f32)
            carryB = cp.tile([128, CB, 1], f32)
            carryG = cp.tile([128, CB, 1], bf16)
            nc.vector.memset(carryA, 0.0)
            nc.vector.memset(carryB, 0.0)
            nc.vector.memset(carryG, 0.0)

            x2d = dramp.tile([NCH, 128, CB, TC], f32)

            def gemm(ps, w_t, co, rhs_hi, rhs_lo=None, w_lo=None, npair=NP):
                csl = slice(co * 128, (co + 1) * 128)
                total = npair * (1 + (rhs_lo is not None) + (w_lo is not None))
                n = 0
                for j in range(npair):
                    wsl = w_t[:, 2 * j:2 * j + 2, csl]
                    rsl = rhs_hi[:, 2 * j:2 * j + 2, :]
                    n += 1
                    nc.tensor.matmul(ps, wsl, rsl, start=(n == 1),
                                     stop=(n == total), perf_mode=DR)
                    if rhs_lo is not None:
                        n += 1
                        nc.tensor.matmul(ps, wsl, rhs_lo[:, 2 * j:2 * j + 2, :],
                                         start=False, stop=(n == total),
                                         perf_mode=DR)
                    if w_lo is not None:
                        n += 1
                        nc.tensor.matmul(ps, w_lo[:, 2 * j:2 * j + 2, csl], rsl,
                                         start=False, stop=(n == total),
                                         perf_mode=DR)

            # ======================= Pass 1: time-mix =======================
            with contextlib.ExitStack() as p1:
                wp = p1.enter_context(tc.tile_pool(name="wp1", bufs=1))
                act = p1.enter_context(tc.tile_pool(name="act1", bufs=2))
                wkv = p1.enter_context(tc.tile_pool(name="wkv", bufs=1))
                ps_mm = p1.enter_context(tc.tile_pool(name="ps1", bufs=6, space="PSUM"))

                wk_t = wp.tile([128, CB, C], f8, tag="wk")
                dmaN(wk_t, Wk8[:, :, :])
                wv_t = wp.tile([128, CB, C], f8, tag="wv")
                dmaN(wv_t, Wv8[:, :, :])
                wr_t = wp.tile([128, CB, C], f8, tag="wr")
                dmaN(wr_t, Wr8[:, :, :])
                woh_t = wp.tile([128, CB, C], f8, tag="woh")
                dmaN(woh_t, Woh[:, :, :])
                wol_t = None
                if SPLIT_WO_W:
                    wol_t = wp.tile([128, CB, C], f8, tag="wol")
                    dmaN(wol_t, Wol[:, :, :])

                for ic in range(NCH):
                    tsl = slice(ic * TC, ic * TC + TC)
                    x_t = act.tile([128, CB, TC], f32, tag="x")
                    dmaN(x_t, xre[:, :, tsl])
                    kh_t = act.tile([128, CB, TC], f8, tag="kh")
                    dmaN(kh_t, ikh.rearrange("(a p) t -> p a t", p=128)[:, :, tsl], 2)
                    vh_t = act.tile([128, CB, TC], f8, tag="vh")
                    dmaN(vh_t, ivh.rearrange("(a p) t -> p a t", p=128)[:, :, tsl], 2)
                    rh_t = act.tile([128, CB, TC], f8, tag="rh")
                    dmaN(rh_t, irh.rearrange("(a p) t -> p a t", p=128)[:, :, tsl], 2)
                    kl_t = vl_t = None
                    if SPLIT_KVR_IN:
                        kl_t = act.tile([128, CB, TC], f8, tag="kl")
                        dmaN(kl_t, ikl.rearrange("(a p) t -> p a t", p=128)[:, :, tsl], 2)
                        vl_t = act.tile([128, CB, TC], f8, tag="vl")
                        dmaN(vl_t, ivl.rearrange("(a p) t -> p a t", p=128)[:, :, tsl], 2)

                    ek = wkv.tile([128, CB, TC], f32, tag="ek")
                    ekv = wkv.tile([128, CB, TC], f32, tag="ekv")
                    th = wkv.tile([128, CB, TC], bf16, tag="th")
                    for co in range(CB):
                        ps_k = ps_mm.tile([128, TC], f32, tag="mm")
                        gemm(ps_k, wk_t, co, kh_t, rhs_lo=kl_t)
                        nc.scalar.activation(out=ek[:, co, :], in_=ps_k,
                                             func=AF.Exp, scale=IWS)
                    for co in range(CB):
                        ps_v = ps_mm.tile([128, TC], f32, tag="mm")
                        gemm(ps_v, wv_t, co, vh_t, rhs_lo=vl_t)
                        nc.vector.scalar_tensor_tensor(
                            out=ekv[:, co, :], in0=ps_v, scalar=IWS,
                            in1=ek[:, co, :], op0=OP.mult, op1=OP.mult)
                    for co in range(CB):
                        ps_r = ps_mm.tile([128, TC], f32, tag="mm")
                        gemm(ps_r, wr_t, co, rh_t)
                        nc.scalar.activation(out=th[:, co, :], in_=ps_r,
                                             func=AF.Tanh, scale=0.5 * IWS)

                    A_t = wkv.tile([128, CB, TC + 1], f32, tag="A")
                    B_t = wkv.tile([128, CB, TC + 1], f32, tag="B")
                    nc.vector.tensor_copy(out=A_t[:, :, 0:1], in_=carryA)
                    nc.vector.tensor_copy(out=B_t[:, :, 0:1], in_=carryB)
                    for cb in range(CB):
                        ewj = ew_t[:, cb:cb + 1]
                        ew_b = bass.AP(tensor=ewj.tensor, offset=ewj.offset,
                                       ap=[ewj.ap[0], [0, TC]])
                        nc.vector.tensor_tensor_scan(
                            out=A_t[:, cb, 1:TC + 1], data0=ew_b,
                            data1=ekv[:, cb, :], initial=A_t[:, cb, 0:1],
                            op0=OP.mult, op1=OP.add)
                        nc.vector.tensor_tensor_scan(
                            out=B_t[:, cb, 1:TC + 1], data0=ew_b,
                            data1=ek[:, cb, :], initial=B_t[:, cb, 0:1],
                            op0=OP.mult, op1=OP.add)
                    nc.vector.tensor_copy(out=carryA, in_=A_t[:, :, TC:TC + 1])
                    nc.vector.tensor_copy(out=carryB, in_=B_t[:, :, TC:TC + 1])

                    num = wkv.tile([128, CB, TC], f32, tag="num")
                    den = wkv.tile([128, CB, TC], f32, tag="den")
                    for cb in range(CB):
                        nc.vector.scalar_tensor_tensor(
                            out=num[:, cb, :], in0=ekv[:, cb, :],
                            scalar=eu_t[:, cb:cb + 1], in1=A_t[:, cb, 0:TC],
                            op0=OP.mult, op1=OP.add)
                        nc.vector.scalar_tensor_tensor(
                            out=den[:, cb, :], in0=ek[:, cb, :],
                            scalar=eu_t[:, cb:cb + 1], in1=B_t[:, cb, 0:TC],
                            op0=OP.mult, op1=OP.add)
                    rden = wkv.tile([128, CB, TC], f32, tag="rden")
                    nc.vector.reciprocal_approx_fast(out=rden, in_=den)
                    t1 = wkv.tile([128, CB, TC], f32, tag="t1")
                    nc.vector.scalar_tensor_tensor(
                        out=t1, in0=th, scalar=1.0, in1=num,
                        op0=OP.add, op1=OP.mult)
                    y8 = wkv.tile([128, CB, TC], f8, tag="y8")
                    nc.vector.tensor_mul(y8, t1, rden)

                    x2_t = pip.tile([128, CB, TC], f32, tag="x2")
                    for co in range(CB):
                        ps_o = ps_mm.tile([128, TC], f32, tag="mm")
                        gemm(ps_o, woh_t, co, y8, w_lo=wol_t)
                        nc.vector.scalar_tensor_tensor(
                            out=x2_t[:, co, :], in0=ps_o, scalar=IWS,
                            in1=x_t[:, co, :], op0=OP.mult, op1=OP.add)
                    dmaN(x2d[ic], x2_t)

            # ========================= Pass 2: FFN ==========================
            with contextlib.ExitStack() as p2:
                wp = p2.enter_context(tc.tile_pool(name="wp2", bufs=1))
                act = p2.enter_context(tc.tile_pool(name="act2", bufs=1))
                op2 = p2.enter_context(tc.tile_pool(name="op2", bufs=1))
                ffn = p2.enter_context(tc.tile_pool(name="ffn", bufs=1))
                pip = p2.enter_context(tc.tile_pool(name="pip", bufs=2))
                rows = p2.enter_context(tc.tile_pool(name="rows", bufs=1))
                ps_f = p2.enter_context(tc.tile_pool(name="ps2", bufs=6, space="PSUM"))
                ps_st = p2.enter_context(tc.tile_pool(name="ps_st", bufs=1, space="PSUM"))
                ps_bc = p2.enter_context(tc.tile_pool(name="ps_bc", bufs=1, space="PSUM"))

                fwkh_t = wp.tile([128, CB, 4 * C], f8, tag="fwkh")
                dmaN(fwkh_t, fWkh[:, :, :], parts=8)
                fwkl_t = None
                if SPLIT_FWK_W:
                    fwkl_t = wp.tile([128, CB, 4 * C], f8, tag="fwkl")
                    dmaN(fwkl_t, fWkl[:, :, :], parts=8)
                fwr_t = wp.tile([128, CB, C], f8, tag="fwr")
                dmaN(fwr_t, fWr8[:, :, :])
                fwvh_t = wp.tile([128, FB, C], f8, tag="fwvh")
                dmaN(fwvh_t, fWvh[:, :, :], parts=8)
                fwvl_t = None
                if SPLIT_FWV_W:
                    fwvl_t = wp.tile([128, FB, C], f8, tag="fwvl")
                    dmaN(fwvl_t, fWvl[:, :, :], parts=8)

                def emit_fwv(pic, kk8, th2, x2_t):
                    ptsl = slice(pic * TC, pic * TC + TC)
                    out_t = op2.tile([128, CB, TC], f32, tag="out")
                    for co in range(CB):
                        ps_v2 = ps_f.tile([128, TC], f32, tag="fz")
                        gemm(ps_v2, fwvh_t, co, kk8, w_lo=fwvl_t, npair=FP)
                        t2 = ffn.tile([128, TC], f32, tag="t2")
                        nc.vector.scalar_tensor_tensor(
                            out=t2, in0=th2[:, co, :], scalar=1.0,
                            in1=ps_v2, op0=OP.add, op1=OP.mult)
                        nc.vector.scalar_tensor_tensor(
                            out=out_t[:, co, :], in0=t2, scalar=IWS,
                            in1=x2_t[:, co, :], op0=OP.mult, op1=OP.add)
                    dmaN(oure[:, :, ptsl], out_t)

                prev = None
                for ic in range(NCH):
                    if prev is not None:
                        emit_fwv(*prev)
                        prev = None
                    tsl = slice(ic * TC, ic * TC + TC)
                    x2_t = pip.tile([128, CB, TC], f32, tag="x2")
                    dmaN(x2_t, x2d[ic])

                    sq = ffn.tile([128, CB, TC], f32r, tag="sq")
                    nc.scalar.activation(out=sq, in_=x2_t, func=AF.Square)
                    st = ps_st.tile([1, 2 * TC], f32, tag="st")
                    x2r = x2_t.bitcast(f32r)
                    for cb in range(CB):
                        nc.tensor.matmul(st[:, 0:TC], ones_k, x2r[:, cb, :],
                                         start=(cb == 0), stop=(cb == CB - 1))
                    for cb in range(CB):
                        nc.tensor.matmul(st[:, TC:2 * TC], ones_k, sq[:, cb, :],
                                         start=(cb == 0), stop=(cb == CB - 1))
                    rw = rows.tile([1, 2 * TC], f32r, tag="rw")
                    rwf = rw.bitcast(f32)
                    tmp = rows.tile([1, 2 * TC], f32, tag="tmp")
                    nc.vector.tensor_scalar_mul(rw[:, 0:TC], st[:, 0:TC], -1.0 / C)
                    nc.vector.tensor_mul(tmp[:, 0:TC], rwf[:, 0:TC], rwf[:, 0:TC])
                    nc.vector.scalar_tensor_tensor(
                        out=tmp[:, TC:2 * TC], in0=st[:, TC:2 * TC],
                        scalar=1.0 / C, in1=tmp[:, 0:TC],
                        op0=OP.mult, op1=OP.subtract)
                    nc.vector.tensor_scalar_add(tmp[:, 0:TC], tmp[:, TC:2 * TC], EPS)
                    nc.vector.reciprocal(out=tmp[:, TC:2 * TC], in_=tmp[:, 0:TC])
                    nc.scalar.activation(out=rw[:, TC:2 * TC],
                                         in_=tmp[:, TC:2 * TC], func=AF.Sqrt)
                    bc2 = ps_bc.tile([128, 2, TC], f32, tag="bc2")
                    mb = bc2[:, 0, :]
                    rb = bc2[:, 1, :]
                    nc.tensor.matmul(mb, ones_b, rw[:, 0:TC])
                    nc.tensor.matmul(rb, ones_b, rw[:, TC:2 * TC])

                    def bmid(ap):
                        return bass.AP(tensor=ap.tensor, offset=ap.offset,
                                       ap=[ap.ap[0], [0, CB], ap.ap[1]])
                    nc.vector.tensor_add(sq, x2_t, bmid(mb))
                    g_t = ffn.tile([128, CB, TC + 1], bf16, tag="g")
                    nc.vector.tensor_copy(out=g_t[:, :, 0:1], in_=carryG)
                    nc.vector.tensor_mul(g_t[:, :, 1:TC + 1], sq.bitcast(f32),
                                         bmid(rb))
                    nc.vector.tensor_copy(out=carryG, in_=g_t[:, :, TC:TC + 1])

                    d2 = ffn.tile([128, CB, TC], bf16, tag="d2")
                    nc.vector.tensor_sub(d2, g_t[:, :, 1:TC + 1], g_t[:, :, 0:TC])
                    inf_b = ffn.tile([128, CB, TC], bf16, tag="infb")
                    infr8 = ffn.tile([128, CB, TC], f8, tag="infr")
                    for cb in range(CB):
                        nc.vector.scalar_tensor_tensor(
                            out=inf_b[:, cb, :], in0=d2[:, cb, :],
                            scalar=ftk_t[:, cb:cb + 1], in1=g_t[:, cb, 0:TC],
                            op0=OP.mult, op1=OP.add)
                        nc.vector.scalar_tensor_tensor(
                            out=infr8[:, cb, :], in0=d2[:, cb, :],
                            scalar=ftr_t[:, cb:cb + 1], in1=g_t[:, cb, 0:TC],
                            op0=OP.mult, op1=OP.add)
                    infh = ffn.tile([128, CB, TC], f8, tag="infh")
                    nc.scalar.activation(out=infh, in_=inf_b, func=AF.Copy)
                    infl = None
                    if SPLIT_FWK_IN:
                        infl = ffn.tile([128, CB, TC], f8, tag="infl")
                        nc.vector.tensor_sub(infl, inf_b, infh)

                    kk8 = ffn.tile([128, FB, TC], f8, tag="kk8")
                    for cog in range(4):
                        rt = ffn.tile([128, CB, TC], bf16, tag="infb")
                        for ci in range(CB):
                            co = cog * CB + ci
                            ps_z = ps_f.tile([128, TC], f32, tag="fz")
                            gemm(ps_z, fwkh_t, co, infh, rhs_lo=infl,
                                 w_lo=fwkl_t)
                            nc.scalar.activation(out=rt[:, ci, :], in_=ps_z,
                                                 func=AF.Relu, scale=IWS,
                                                 bias=bfk_t[:, co:co + 1])
                        nc.vector.tensor_mul(
                            kk8[:, cog * CB:(cog + 1) * CB, :], rt, rt)

                    th2 = ffn.tile([128, CB, TC], bf16, tag="th2")
                    for co in range(CB):
                        ps_r2 = ps_f.tile([128, TC], f32, tag="fz")
                        gemm(ps_r2, fwr_t, co, infr8)
                        nc.scalar.activation(out=th2[:, co, :], in_=ps_r2,
                                             func=AF.Tanh, scale=0.5 * IWS,
                                             bias=bfr_t[:, co:co + 1])

                    prev = (ic, kk8, th2, x2_t)
                emit_fwv(*prev)

    nc.finalize()
    return nc


def _q8(x, s=1.0):
    return (np.asarray(x, np.float32) * s).astype(E4)


def _pack_w(W):
    """[C, M] -> [128, CB, M] with a = c // 128, p = c % 128."""
    Cin, M = W.shape
    return np.ascontiguousarray(W.reshape(Cin // 128, 128, M).transpose(1, 0, 2))


def _prep_maps(inputs):
    x = np.asarray(inputs["x"], np.float32)
    ln1_g = np.asarray(inputs["ln1_g"], np.float32)
    ln1_b = np.asarray(inputs["ln1_b"], np.float32)
    ln2_g = np.asarray(inputs["ln2_g"], np.float32)
    ln2_b = np.asarray(inputs["ln2_b"], np.float32)
    tmk = np.asarray(inputs["tmk"], np.float32)
    tmv = np.asarray(inputs["tmv"], np.float32)
    tmr = np.asarray(inputs["tmr"], np.float32)
    ftmk = np.asarray(inputs["ftmk"], np.float32)
    ftmr = np.asarray(inputs["ftmr"], np.float32)

    m = x.mean(-1, keepdims=True)
    v = np.square(x - m).mean(-1, keepdims=True)
    h = ((x - m) / np.sqrt(v + EPS)) * ln1_g + ln1_b
    hh = np.concatenate([np.zeros((B, 1, C), np.float32), h[:, :-1]], 1)
    ink = h * tmk + hh * (1 - tmk)
    inv = h * tmv + hh * (1 - tmv)
    inr = h * tmr + hh * (1 - tmr)

    def split(t):
        hi = _q8(t)
        lo = _q8(t - hi.astype(np.float32))
        return hi, lo

    ikh_, ikl_ = split(ink)
    ivh_, ivl_ = split(inv)
    irh_ = _q8(inr)

    Wk = np.asarray(inputs["Wk"], np.float32)
    Wv = np.asarray(inputs["Wv"], np.float32) * 0.5
    Wr = np.asarray(inputs["Wr"], np.float32)
    Wo = np.asarray(inputs["Wo"], np.float32)
    fWk = np.asarray(inputs["fWk"], np.float32) * ln2_g[:, None]
    fWr = np.asarray(inputs["fWr"], np.float32) * ln2_g[:, None]
    fWv = np.asarray(inputs["fWv"], np.float32) * 0.5

    def wsplit(W):
        hi = _q8(W, WS)
        lo = _q8(W - hi.astype(np.float32) / WS, WS)
        return hi, lo

    Woh_, Wol_ = wsplit(Wo)
    fWkh_, fWkl_ = wsplit(fWk)
    fWvh_, fWvl_ = wsplit(fWv)

    ew = np.exp(-np.exp(np.asarray(inputs["time_decay"], np.float32)))
    eu = np.exp(np.asarray(inputs["time_first"], np.float32))
    bias_fk = ln2_b @ np.asarray(inputs["fWk"], np.float32)
    bias_fr = ln2_b @ np.asarray(inputs["fWr"], np.float32)

    def rows128(a):
        return np.ascontiguousarray(a.reshape(-1, 128).T.astype(np.float32))

    common = {
        "Wk8": _pack_w(_q8(Wk, WS)), "Wv8": _pack_w(_q8(Wv, WS)),
        "Wr8": _pack_w(_q8(Wr, WS)),
        "Woh": _pack_w(Woh_), "Wol": _pack_w(Wol_),
        "fWkh": _pack_w(fWkh_), "fWkl": _pack_w(fWkl_),
        "fWr8": _pack_w(_q8(fWr, WS)),
        "fWvh": _pack_w(fWvh_), "fWvl": _pack_w(fWvl_),
        "euT": rows128(eu), "ewT": rows128(ew),
        "ftkT": rows128(ftmk), "ftrT": rows128(ftmr),
        "bfrT": rows128(0.5 * bias_fr), "bfkT": rows128(bias_fk),
        "ones128": np.ones(128, np.float32),
    }
    maps = []
    for b in range(B):
        maps.append({**common,
                     "xT": np.ascontiguousarray(x[b].T),
                     "ikh": np.ascontiguousarray(ikh_[b].T),
                     "ikl": np.ascontiguousarray(ikl_[b].T),
                     "ivh": np.ascontiguousarray(ivh_[b].T),
                     "ivl": np.ascontiguousarray(ivl_[b].T),
                     "irh": np.ascontiguousarray(irh_[b].T)})
    return maps


def get_nc():
    if "nc" not in _CACHE:
        _CACHE["nc"] = _build()
    return _CACHE["nc"]


def kernel(**inputs):
    from concourse.bass_utils import run_bass_kernel_spmd
    nc = get_nc()
    in_maps = _prep_maps(inputs)
    res = run_bass_kernel_spmd(nc, in_maps, core_ids=list(range(B)))
    return np.stack([np.ascontiguousarray(r["outT"].T) for r in res.results])
